# revision 11
# baseline (speedup 1.0000x reference)
"""Conv2D 3x3 stride-1 pad-1 (NCHW) on 8 NeuronCores via 1D Winograd F(2,3).

Strategy: data-parallel over batch (32 imgs -> 4 per core). Winograd F(2,3)
along H cuts tensor-engine work 1.5x vs direct implicit GEMM: per output
row-pair only 4 winograd components x 3 width-taps = 12 matmul rows feed
2 output rows (vs 18 direct). All matmul traffic is bf16 (rel err ~5e-3,
gate 2e-2).

Per image (padded rows 0..57, host-padded, bf16):
  d_a = x[a::2] (28 rows each), a=0..3
  V0 = d0-d2, V1 = d1+d2, V2 = d2-d1, V3 = d1-d3
  M[i] = sum_kw W'[i,kw]^T V[i][:, ty, kw:kw+56]        (PE, 3-tap PSUM acc)
  z0 = M0+M1+M2+b -> even rows, z1 = M1-M2-M3+b -> odd rows
W'[i,kw][c,o] = sum_kh G[i,kh] w[o,c,kh,kw] computed on host fp32 -> bf16.
Output written bf16, host upcasts.

Engine budget per ty-group (PE: 12 matmuls = 2.0us): fixed per-op costs
dominate at this tile size, so ops are batched:
  - all 4 M planes land in ONE 4-bank PSUM tile; eviction to bf16 SBUF is
    ONE ScalarE activation per group (~1.3us).
  - output transform (4 plain tensor_tensor) + bias (one 4x-mode
    tensor_scalar over the row-interleaved out tile) run once per TWO
    groups on DVE (~1.4us/group amortized).
  - V transforms for images 1-3 run on the otherwise-idle GpSimd engine,
    emitted a full image ahead so its ~0.45 el/ns keeps up.
ty (28 row-pairs) in 4 groups of 7 -> matmul free dim 392; two groups in
flight across the 8 PSUM banks.
"""

import os
import sys

import numpy as np

if "/opt/trn_rl_repo" not in sys.path:
    sys.path.insert(0, "/opt/trn_rl_repo")

from concourse import bacc, bass, mybir  # noqa: E402
from concourse.bass_utils import run_bass_kernel_spmd  # noqa: E402
from concourse.tile import TileContext, add_dep_helper  # noqa: E402

N_FULL, CIN, H, W = 32, 128, 56, 56
COUT = 256
NCORES = 8
NPER = N_FULL // NCORES  # 4 images per core
HP, WP = H + 2, W + 2  # 58 x 58 padded
NI = 4  # winograd components
KWT = 3  # width taps
TY = H // 2  # 28 output row-pairs
GTY = 7  # row-pairs per matmul group
NG = TY // GTY  # 4 groups
NFREE = GTY * W  # 392 moving free dim
OCH = COUT // 128  # 2 output-channel chunks

_CACHE = {}


def _build_conv(with_bias):
    f32 = mybir.dt.float32
    bf16 = mybir.dt.bfloat16

    nc = bacc.Bacc(None, target_bir_lowering=False)

    x_par = nc.declare_dram_parameter("x", [NPER, CIN, HP, WP], bf16, isOutput=False)
    w_par = nc.declare_dram_parameter(
        "wt", [CIN, OCH * NI * KWT * 128], bf16, isOutput=False
    )
    bias_par = nc.declare_dram_parameter("bias", [COUT], f32, isOutput=False)
    out_par = nc.declare_dram_parameter("out", [NPER, COUT, H, W], bf16, isOutput=True)
    out_flat = out_par.rearrange("n o h w -> n o (h w)")
    # dram weight view: [cin, oc, (i kw), o_in]
    w_dram = w_par.rearrange("p (c t o) -> p c t o", c=OCH, t=NI * KWT)

    with TileContext(nc) as tc:
        with (
            tc.tile_pool(name="const", bufs=1) as cpool,
            tc.tile_pool(name="xin", bufs=1) as xpool,
            tc.tile_pool(name="vpl", bufs=1) as vpool,
            tc.tile_pool(name="psum", bufs=2, space="PSUM") as ppool,
            tc.tile_pool(name="mev", bufs=4) as mpool,
            tc.tile_pool(name="outp", bufs=4) as opool,
        ):
            # SBUF tiles: all 4 images + their winograd planes stay resident.
            xts = [
                xpool.tile([CIN, HP, WP], bf16, tag=f"x{n}", name="x")
                for n in range(NPER)
            ]
            vts = [
                vpool.tile([CIN, NI, TY, HP], bf16, tag=f"v{n}", name="v")
                for n in range(NPER)
            ]
            w_sb = cpool.tile([CIN, OCH, NI * KWT, 128], bf16, tag="w", name="w")
            bias_sb = cpool.tile([128, OCH], f32, tag="bias")

            # Head DMAs. The sync queue is live ~6us before the
            # gpsimd/scalar queues (engine init), so everything the first
            # image pass needs goes there, interleaved by criticality.
            XSPL = 31
            nc.sync.dma_start(out=xts[0][:, 0:XSPL, :], in_=x_par[0][:, 0:XSPL, :])
            nc.sync.dma_start(out=w_sb[:, 0], in_=w_dram[:, 0])
            nc.sync.dma_start(out=xts[0][:, XSPL:HP, :], in_=x_par[0][:, XSPL:HP, :])
            nc.sync.dma_start(
                out=bias_sb[:], in_=bias_par.rearrange("(a b) -> b a", b=128)
            )
            nc.sync.dma_start(out=w_sb[:, 1], in_=w_dram[:, 1])
            for n in range(1, NPER):
                nc.gpsimd.dma_start(
                    out=xts[n][:, 0:XSPL, :], in_=x_par[n][:, 0:XSPL, :]
                )
                nc.scalar.dma_start(
                    out=xts[n][:, XSPL:HP, :], in_=x_par[n][:, XSPL:HP, :]
                )

            def v_transform(n, t0, t1, eng03=None):
                """Emit ops computing V planes for image n, ty range [t0,t1).
                V1/V2 go on DVE; V0/V3 on eng03 (default DVE)."""
                xv = xts[n].rearrange("p (hh two) w -> p two hh w", two=2)
                v = vts[n]
                d0 = xv[:, 0, t0:t1, :]
                d1 = xv[:, 1, t0:t1, :]
                d2 = xv[:, 0, t0 + 1 : t1 + 1, :]
                d3 = xv[:, 1, t0 + 1 : t1 + 1, :]
                e03 = eng03 or nc.vector
                e03.tensor_sub(v[:, 0, t0:t1, :], d0, d2)
                nc.vector.tensor_add(v[:, 1, t0:t1, :], d1, d2)
                nc.vector.tensor_sub(v[:, 2, t0:t1, :], d2, d1)
                e03.tensor_sub(v[:, 3, t0:t1, :], d1, d3)

            # Image 0 on DVE (idle during head), in two chunks so matmuls
            # start after the first x row-half lands.
            v_transform(0, 0, 14)
            v_transform(0, 14, TY)

            for n in range(NPER):
                for oc in range(OCH):
                    for g2 in range(NG // 2):  # pairs of ty-groups
                        ot = opool.tile([128, 2, GTY, 2, W], bf16, tag="ot", name="ot")
                        for gp in range(2):
                            g = g2 * 2 + gp
                            ps = ppool.tile([128, NI, 512], f32, tag="ps", name="ps")
                            for i in range(NI):
                                for kw in range(KWT):
                                    nc.tensor.matmul(
                                        ps[:, i, 0:NFREE],
                                        w_sb[:, oc, i * KWT + kw, :],
                                        vts[n][
                                            :, i, g * GTY : (g + 1) * GTY, kw : kw + W
                                        ],
                                        start=(kw == 0),
                                        stop=(kw == KWT - 1),
                                    )
                            # ONE eviction per group: 4 planes, PSUM->bf16
                            mg = mpool.tile([128, NI, NFREE], bf16, tag="m", name="m")
                            nc.scalar.copy(mg[:], ps[:, :, 0:NFREE])
                            # output transform, flat 2D ops (fastest DVE mode)
                            tt = mpool.tile([128, NFREE], bf16, tag="t", name="t")
                            ut = mpool.tile([128, NFREE], bf16, tag="u", name="u")
                            m0, m1, m2, m3 = (mg[:, i, :] for i in range(NI))

                            def wv(a):  # [128, NFREE] -> [128, GTY, W]
                                return a.rearrange("p (a w) -> p a w", w=W)

                            if with_bias:
                                po = opool.tile(
                                    [128, GTY, 2, W], bf16, tag="po", name="po"
                                )
                                z0, z1 = po[:, :, 0, :], po[:, :, 1, :]
                            else:
                                z0, z1 = ot[:, gp, :, 0, :], ot[:, gp, :, 1, :]
                            nc.vector.tensor_add(tt[:], m0, m1)
                            nc.vector.tensor_add(z0, wv(tt), wv(m2))
                            nc.vector.tensor_sub(ut[:], m1, m2)
                            nc.vector.tensor_sub(z1, wv(ut), wv(m3))
                            if with_bias:
                                nc.vector.tensor_scalar_add(
                                    ot[:, gp].rearrange("p a b w -> p (a b w)"),
                                    po.rearrange("p a b w -> p (a b w)"),
                                    bias_sb[:, oc : oc + 1],
                                )
                        nc.sync.dma_start(
                            out=out_flat[
                                n,
                                oc * 128 : (oc + 1) * 128,
                                g2 * (4 * NFREE) : (g2 + 1) * (4 * NFREE),
                            ],
                            in_=ot.rearrange("p g a b w -> p (g a b w)"),
                        )
                        # next image's V transform, a full image ahead
                        # (emitted during oc=0 so it has ~16us of room);
                        # V0/V3 go to the otherwise-idle GpSimd engine.
                        if oc == 0 and n + 1 < NPER:
                            v_transform(
                                n + 1, g2 * 2 * GTY, (g2 + 1) * 2 * GTY, nc.gpsimd
                            )
    nc.compile()
    return nc


def _get_nc(with_bias):
    key = ("wino", with_bias)
    if key not in _CACHE:
        _CACHE[key] = _build_conv(with_bias)
    return _CACHE[key]


# test-harness hooks: set TRACE=True before calling kernel() to capture an
# NTFF profile; LAST_RESULTS then holds the BassKernelResults.
TRACE = False
LAST_RESULTS = None
MODE = "wino-bf16-v4"

# F(2,3) filter transform
_G = np.array(
    [[1.0, 0.0, 0.0], [0.5, 0.5, 0.5], [0.5, -0.5, 0.5], [0.0, 0.0, 1.0]],
    dtype=np.float64,
)


def kernel(x, weight, bias):
    global LAST_RESULTS
    import ml_dtypes

    bfl = ml_dtypes.bfloat16

    x = np.ascontiguousarray(np.asarray(x), dtype=np.float32)
    w = np.ascontiguousarray(np.asarray(weight), dtype=np.float32)
    b = np.ascontiguousarray(np.asarray(bias), dtype=np.float32)

    xp = np.pad(x, ((0, 0), (0, 0), (1, 1), (1, 1))).astype(bfl)
    # W'[i, c, kw, o] = sum_kh G[i,kh] w[o,c,kh,kw] -> layout [c, (oc i kw o_in)]
    wp = np.einsum("ik,ockl->iclo", _G, w.astype(np.float64))  # i, c, kw, o
    wt = (
        wp.transpose(1, 3, 0, 2)  # c, o, i, kw
        .reshape(CIN, OCH, 128, NI, KWT)
        .transpose(0, 1, 3, 4, 2)  # c, oc, i, kw, o_in
        .reshape(CIN, OCH * NI * KWT * 128)
    )
    wt = np.ascontiguousarray(wt.astype(np.float32)).astype(bfl)

    per_core = [
        {"x": xp[c * NPER : (c + 1) * NPER], "wt": wt, "bias": b}
        for c in range(NCORES)
    ]

    kwargs = {}
    if TRACE:
        kwargs = dict(trace=True, trace_cores=[0])
    res = run_bass_kernel_spmd(
        _get_nc(with_bias=bool(np.any(b != 0.0))),
        per_core,
        core_ids=list(range(NCORES)),
        **kwargs,
    )
    LAST_RESULTS = res
    return np.concatenate([r["out"] for r in res.results], axis=0).astype(np.float32)
